# revision 17
# baseline (speedup 1.0000x reference)
"""BERT layer forward (nn_BertLayerForDecoder) on 8 trn2 NeuronCores.

Sharding: sequence-parallel. The (B=2, S=2048) = 4096 token rows are split
into 8 slices of 512 rows; core c owns rows [r*512, (r+1)*512) of batch
b = c // 4, r = c % 4. Q and V are projected per-slice; V is AllGathered
within each 4-core batch group (1 MB/rank). K^T is computed fully
per-core (each core holds the whole batch-slice key input): the own-block
K projection runs first so a "local pass" of attention over the own 512
keys starts immediately, and the remaining 3 blocks' K projections are
exactly the compute that hides the V gather. K^T stays resident in SBUF
(no HBM roundtrip). Key chunks are processed in rotation order (own
block first); the attention mask and the key input are host-permuted to
match, and the gathered V blocks are addressed with partition_id()-based
dynamic DMA offsets.

Softmax: no max-subtraction (scores are O(1)); mask add + 1/sqrt(dh)
scale folded into the exp. The exp work is split between the ACT engine
(exact spline exp) and the Vector engine (Schraudolph exponent-stuffing:
one fused multiply-add against a 2^23*1.5 anchor leaves the bf16 bit
pattern of exp(x) in the low fp32 mantissa bits, extracted with a
strided u16 copy). The softmax denominator comes from ones-columns
appended to V, so it falls out of the ctx matmul.

Numerics: matmul operands bf16 (fp32 PSUM accumulation), vector math in
fp32. DVE-exp probs carry ~3% relative error; the attention branch is
~1% of the output signal (the residual dominates), so the end-to-end
impact is ~1e-4.

Self-contained: hardcodes all shapes; only needs numpy + ml_dtypes + the
installed concourse package.
"""

import ml_dtypes
import numpy as np

import concourse.bacc as bacc
import concourse.bass as cbass
import concourse.mybir as mybir
import concourse.tile as tile
from concourse.bass_utils import run_bass_kernel_spmd
from concourse.masks import make_identity

F32 = mybir.dt.float32
BF16 = mybir.dt.bfloat16
U16 = mybir.dt.uint16
AF = mybir.ActivationFunctionType
OP = mybir.AluOpType
NPBF = ml_dtypes.bfloat16

B, S, D, H, DH, DFF = 2, 2048, 1024, 16, 64, 4096
P = 128
NQ = 512              # query rows per core
QC = NQ // P          # 4 q-chunks
KC = D // P           # 8 d-chunks (contraction)
SC = S // P           # 16 key chunks
FC = DFF // P         # 32 dff chunks
DG = FC // 4          # 8 ffn-up column groups (512 cols each)
EPS = 1e-12
KV_V = NQ * D          # bf16 elements gathered per rank (V rows)

# Schraudolph exp in bf16-bit space: with t = s*C_EXP + (mask*C_LOG2E128
# + C_BIAS + C_ANCHOR), the low 16 bits of fp32(t) hold bf16(exp(s*0.125
# + mask)). C_EXP folds the 1/sqrt(dh) attention scale.
C_LOG2E128 = 128 * 1.4426950408889634
C_EXP = 0.125 * C_LOG2E128
C_BIAS = 16256.0 - 5.590656            # 128*(127 - 0.043677)
C_ANCHOR = 12582912.0                  # 2^23 * 1.5
# which chunk indices of each pair-iteration run exp on the DVE (rest
# on ACT); tuned so the two engines finish together.
DVE_SC_LOCAL = (1, 3)
DVE_SC_REMOTE = (1, 4, 7, 10)

_CACHE = {}


def _build():
    nc = bacc.Bacc()

    # activations (own 512-row slice, pre-transposed bf16), except the
    # key input which is the full 2048-row batch slice in rotation order
    xqT = nc.declare_dram_parameter("xqT", [P, KC, NQ], BF16, isOutput=False)
    xkTF = nc.declare_dram_parameter("xkTF", [P, KC, S], BF16,
                                     isOutput=False)
    xvT = nc.declare_dram_parameter("xvT", [P, KC, NQ], BF16, isOutput=False)
    xq = nc.declare_dram_parameter("xq", [NQ, D], F32, isOutput=False)
    # mask/magic host-permuted into this core's rotation chunk order
    msk = nc.declare_dram_parameter("mask", [S], F32, isOutput=False)
    mgk = nc.declare_dram_parameter("magicm", [S], F32, isOutput=False)
    # weights: bf16, pre-tiled
    WqT = nc.declare_dram_parameter("WqT", [P, KC, D], BF16, isOutput=False)
    WkT = nc.declare_dram_parameter("WkT", [P, KC, D], BF16, isOutput=False)
    WvT = nc.declare_dram_parameter("WvT", [P, KC, D], BF16, isOutput=False)
    WoT = nc.declare_dram_parameter("WoT", [P, KC, D], BF16, isOutput=False)
    WiT = nc.declare_dram_parameter("WiT", [DG, P, KC, NQ], BF16,
                                    isOutput=False)
    WdT = nc.declare_dram_parameter("WdT", [P, FC, D], BF16, isOutput=False)
    bq = nc.declare_dram_parameter("bq", [D], F32, isOutput=False)
    bk = nc.declare_dram_parameter("bk", [D], F32, isOutput=False)
    bv = nc.declare_dram_parameter("bv", [D], F32, isOutput=False)
    bo = nc.declare_dram_parameter("bo", [D], F32, isOutput=False)
    bi = nc.declare_dram_parameter("bi", [DFF], F32, isOutput=False)
    bd = nc.declare_dram_parameter("bd", [D], F32, isOutput=False)
    g1 = nc.declare_dram_parameter("ln1_g", [D], F32, isOutput=False)
    b1 = nc.declare_dram_parameter("ln1_b", [D], F32, isOutput=False)
    g2 = nc.declare_dram_parameter("ln2_g", [D], F32, isOutput=False)
    b2 = nc.declare_dram_parameter("ln2_b", [D], F32, isOutput=False)
    out = nc.declare_dram_parameter("out", [NQ, D], F32, isOutput=True)

    # V collective bounce buffers (bf16); per block V is [NQ, D] s-major
    kvLb = nc.dram_tensor("kv_loc", [KV_V], BF16)
    kvAb = nc.dram_tensor("kv_all", [4 * KV_V], BF16)
    kvL_v = kvLb[:].rearrange("(s d) -> s d", d=D)

    with tile.TileContext(nc) as tc:
        with tc.tile_pool(name="const", bufs=1) as const:
            # ---------- small constants (resident) ----------
            mask_sb = const.tile([P, SC], F32)
            nc.gpsimd.dma_start(mask_sb, msk.rearrange("(c p) -> p c", p=P))
            magic_sb = const.tile([P, SC], F32)
            nc.gpsimd.dma_start(magic_sb, mgk.rearrange("(c p) -> p c", p=P))
            bq_p = const.tile([P, KC], F32)
            nc.gpsimd.dma_start(bq_p, bq.rearrange("(c p) -> p c", p=P))
            bk_p = const.tile([P, KC], F32)
            nc.gpsimd.dma_start(bk_p, bk.rearrange("(c p) -> p c", p=P))
            bi_p = const.tile([P, FC], F32)
            nc.gpsimd.dma_start(bi_p, bi.rearrange("(c p) -> p c", p=P))
            eps_sb = const.tile([P, 1], F32)
            nc.vector.memset(eps_sb, EPS)
            bv_r = const.tile([P, D], F32)
            nc.gpsimd.dma_start(bv_r,
                                bv.ap().unsqueeze(0).to_broadcast((P, D)))

            def rep_row(pool, vec, name):
                t = pool.tile([P, D], F32, tag=name, name=name)
                nc.sync.dma_start(t, vec.ap().unsqueeze(0).to_broadcast((P, D)))
                return t

            def layernorm(pool, x_res, qc, g_r, b_r, dst_ap, sfx):
                """mean/var via bn_stats; the [P, D] affine passes split
                across gpsimd/vector so neither serializes the tail."""
                st6 = pool.tile([P, 2, 6], F32, tag="st6" + sfx, name="st6")
                for j in range(2):
                    nc.vector.bn_stats(
                        st6[:, j, :], x_res[:, qc, j * 512:(j + 1) * 512])
                mv = pool.tile([P, 2], F32, tag="mv" + sfx, name="mv")
                nc.vector.bn_aggr(mv, st6)
                sq = pool.tile([P, 1], F32, tag="sq" + sfx, name="sq")
                nc.scalar.activation(sq, mv[:, 1:2], AF.Sqrt, bias=eps_sb)
                rstd = pool.tile([P, 1], F32, tag="rstd" + sfx, name="rstd")
                nc.vector.reciprocal(rstd, sq)
                xn = pool.tile([P, D], F32, tag="xn" + sfx, name="xn")
                nc.gpsimd.tensor_scalar(
                    xn, x_res[:, qc, :], mv[:, 0:1], rstd,
                    OP.subtract, OP.mult)
                xg = pool.tile([P, D], F32, tag="xg" + sfx, name="xg")
                nc.gpsimd.tensor_tensor(xg, xn, g_r, OP.mult)
                nc.vector.tensor_tensor(dst_ap, xg, b_r, OP.add)

            pCD_cm = tc.tile_pool(name="pCD", bufs=1)
            pCD = pCD_cm.__enter__()
            attn1 = pCD.tile([P, QC, D], F32)      # LN1 out (residual)
            attn1T = pCD.tile([P, KC, NQ], BF16)
            persBC_cm = tc.tile_pool(name="persBC", bufs=1)
            persBC = persBC_cm.__enter__()
            ctxT = persBC.tile([P, KC, NQ], BF16)      # ctx^T (dh-pairs, q)
            wo_b = persBC.tile([P, KC, D], BF16)       # Wo (loaded early)

            with tc.tile_pool(name="persB", bufs=1) as persB:
                QT = persB.tile([P, KC, NQ], BF16)     # Q^T
                kT_all = persB.tile([P, KC, S], BF16)  # all keys^T, resident
                Vs = persB.tile([P, SC, H, DH + 2], BF16)  # V + ones cols
                ctx_acc = persB.tile([P, KC, 2, NQ], BF16)  # local-pass ctx

                # ======== phase A: projections; V gather ========
                with (
                    tc.tile_pool(name="xT", bufs=2) as xT,
                    tc.tile_pool(name="wA", bufs=2) as wA,
                    tc.tile_pool(name="vsbA", bufs=1) as vsbA,
                    tc.tile_pool(name="psA", bufs=1, space="PSUM") as psA,
                ):
                    # --- own-block K projection (kc-major) ---
                    keyT = xT.tile([P, KC, NQ], BF16, tag="xpt", name="keyT")
                    wk_b = wA.tile([P, KC, D], BF16, tag="wk", name="wk_b",
                                   bufs=1)
                    for kk in range(0, KC, 2):
                        nc.sync.dma_start(keyT[:, kk:kk + 2, :],
                                          xkTF[:, kk:kk + 2, 0:NQ])
                        nc.sync.dma_start(wk_b[:, kk:kk + 2, :],
                                          WkT[:, kk:kk + 2, :])
                    ppK = [psA.tile([P, NQ], F32, tag=f"psA{j}",
                                    name=f"ppK{j}") for j in range(KC)]
                    for kc in range(KC):
                        for dc in range(KC):
                            nc.tensor.matmul(
                                ppK[dc], wk_b[:, kc, dc * P:(dc + 1) * P],
                                keyT[:, kc, :],
                                start=(kc == 0), stop=(kc == KC - 1))
                    for dc in range(KC):
                        nc.scalar.add(kT_all[:, dc, 0:NQ], ppK[dc],
                                      bk_p[:, dc:dc + 1])

                    # --- V projection -> gather ---
                    v_sb = vsbA.tile([P, QC, D], BF16)
                    valT = xT.tile([P, KC, NQ], BF16, tag="xpt", name="valT")
                    wv_b = wA.tile([P, KC, D], BF16, tag="wv", name="wv_b",
                                   bufs=1)
                    for kk in range(0, KC, 2):
                        nc.sync.dma_start(valT[:, kk:kk + 2, :],
                                          xvT[:, kk:kk + 2, :])
                        nc.sync.dma_start(wv_b[:, kk:kk + 2, :],
                                          WvT[:, kk:kk + 2, :])
                    ppV = [psA.tile([P, NQ], F32, tag=f"psA{j}",
                                    name=f"ppV{j}") for j in range(KC)]
                    for kc in range(KC):
                        for sl in range(KC):
                            sc4, hf = sl // 2, sl % 2
                            nc.tensor.matmul(
                                ppV[sl], valT[:, kc, sc4 * P:(sc4 + 1) * P],
                                wv_b[:, kc, hf * 512:(hf + 1) * 512],
                                start=(kc == 0), stop=(kc == KC - 1))
                    for sl in range(KC):
                        sc4, hf = sl // 2, sl % 2
                        nc.vector.tensor_tensor(
                            v_sb[:, sc4, hf * 512:(hf + 1) * 512], ppV[sl],
                            bv_r[:, hf * 512:(hf + 1) * 512], OP.add)
                    for sc4 in range(QC):
                        nc.scalar.dma_start(
                            kvL_v[sc4 * P:(sc4 + 1) * P, :], v_sb[:, sc4, :])

                    nc.gpsimd.collective_compute(
                        "AllGather", OP.bypass,
                        replica_groups=[[0, 1, 2, 3], [4, 5, 6, 7]],
                        ins=[kvLb[:]], outs=[kvAb[:]])

                    # --- Q projection (overlaps the gather) ---
                    qryT = xT.tile([P, KC, NQ], BF16, tag="xpt", name="qryT")
                    wq_b = wA.tile([P, KC, D], BF16, tag="wv", name="wq_b",
                                   bufs=1)
                    for kk in range(0, KC, 2):
                        nc.sync.dma_start(qryT[:, kk:kk + 2, :],
                                          xqT[:, kk:kk + 2, :])
                        nc.sync.dma_start(wq_b[:, kk:kk + 2, :],
                                          WqT[:, kk:kk + 2, :])
                    ppQ = [psA.tile([P, NQ], F32, tag=f"psA{j}",
                                    name=f"ppQ{j}") for j in range(KC)]
                    for kc in range(KC):
                        for dc in range(KC):
                            nc.tensor.matmul(
                                ppQ[dc], wq_b[:, kc, dc * P:(dc + 1) * P],
                                qryT[:, kc, :],
                                start=(kc == 0), stop=(kc == KC - 1))
                    for dc in range(KC):
                        nc.vector.tensor_scalar_add(
                            QT[:, dc, :], ppQ[dc], bq_p[:, dc:dc + 1])

                    # own V rows -> Vs rotation slots 0..3 (zero DMA)
                    nc.gpsimd.memset(Vs[:, :, :, DH:DH + 2], 1.0)
                    for c in range(QC):
                        nc.vector.tensor_copy(
                            Vs[:, c, :, 0:DH],
                            v_sb[:, c, :].rearrange("p (h dh) -> p h dh",
                                                    dh=DH))

                    # Wo prefetch (used in phase C)
                    for kk in range(0, KC, 4):
                        nc.sync.dma_start(wo_b[:, kk:kk + 4, :],
                                          WoT[:, kk:kk + 4, :])

                    # --- remote-block K projections: the gather cover ---
                    for bi2 in range(3):
                        keyR = xT.tile([P, KC, NQ], BF16, tag="xpt",
                                       name="keyR")
                        col = (1 + bi2) * NQ
                        for kk in range(0, KC, 2):
                            nc.sync.dma_start(
                                keyR[:, kk:kk + 2, :],
                                xkTF[:, kk:kk + 2, col:col + NQ])
                        ppR = [psA.tile([P, NQ], F32, tag=f"psA{j}",
                                        name=f"ppR{j}") for j in range(KC)]
                        for kc in range(KC):
                            for dc in range(KC):
                                nc.tensor.matmul(
                                    ppR[dc],
                                    wk_b[:, kc, dc * P:(dc + 1) * P],
                                    keyR[:, kc, :],
                                    start=(kc == 0), stop=(kc == KC - 1))
                        for dc in range(KC):
                            nc.scalar.add(kT_all[:, dc, col:col + NQ],
                                          ppR[dc], bk_p[:, dc:dc + 1])

                # ======== phase B: attention ========
                def emit_exp(probs_t, ci, sp, col, on_dve, scratch):
                    """exp of [P, 2, NQ] scores -> bf16 probs chunk ci of
                    the u16 probs tile [P, n, 2, NQ]."""
                    if on_dve:
                        texp = scratch.tile([P, 2, NQ], F32, tag="texp",
                                            name="texp")
                        nc.vector.tensor_scalar(
                            texp, sp, C_EXP, magic_sb[:, col:col + 1],
                            OP.mult, OP.add)
                        tv = texp[:, :, :].bitcast(U16).rearrange(
                            "p h (q t) -> p t h q", t=2)
                        nc.vector.tensor_copy(
                            probs_t[:, ci:ci + 1, :, :], tv[:, 0:1, :, :])
                    else:
                        nc.scalar.activation(
                            probs_t[:, ci, :, :].bitcast(BF16), sp, AF.Exp,
                            bias=mask_sb[:, col:col + 1], scale=0.125)

                with (
                    tc.tile_pool(name="texpp", bufs=1) as texpp,
                    tc.tile_pool(name="smallB", bufs=1) as smallB,
                    tc.tile_pool(name="ps_sc", bufs=2, space="PSUM") as ps_sc,
                    tc.tile_pool(name="ps_ctx", bufs=2,
                                 space="PSUM") as ps_ctx,
                ):
                    # ---- local pass: own 4 key chunks ----
                    with tc.tile_pool(name="probsL", bufs=2) as probsL:
                        for pair in range(H // 2):
                            probs = probsL.tile([P, QC, 2, NQ], U16,
                                                tag="pl", name="probs")
                            for c in range(QC):
                                sp = ps_sc.tile([P, 2, NQ], F32, tag="sp",
                                                name="sp")
                                for i in range(2):
                                    nc.tensor.matmul(
                                        sp[:, i, :],
                                        kT_all[i * DH:(i + 1) * DH, pair,
                                               c * P:(c + 1) * P],
                                        QT[i * DH:(i + 1) * DH, pair, :],
                                        start=True, stop=True)
                                emit_exp(probs, c, sp, c,
                                         c in DVE_SC_LOCAL, texpp)
                            cp = ps_ctx.tile([P, 2, NQ], F32, tag="cp",
                                             name="cp")
                            for c in range(QC):
                                for i in range(2):
                                    nc.tensor.matmul(
                                        cp[0:DH + 2, i, :],
                                        Vs[:, c, 2 * pair + i, :],
                                        probs[:, c, i, :].bitcast(BF16),
                                        start=(c == 0), stop=(c == QC - 1))
                            nc.vector.tensor_copy(
                                ctx_acc[0:DH + 1, pair, :, :],
                                cp[0:DH + 1, :, :])

                    # ---- remote Vs assembly (rotation order) ----
                    own_s = nc.sync.partition_id()
                    blk_s = [nc.sync.snap(((own_s & 3) + 1 + bi) & 3,
                                          min_val=0, max_val=3)
                             for bi in range(3)]
                    with (
                        tc.tile_pool(name="vstr", bufs=2) as vstr,
                        tc.tile_pool(name="probsR", bufs=3) as probsR,
                    ):
                        for bi in range(3):
                            for c in range(QC):
                                vt = vstr.tile([P, D], BF16, tag="vstr",
                                               name="vt")
                                off = blk_s[bi] * KV_V + c * P * D
                                nc.sync.dma_start(
                                    vt,
                                    kvAb[cbass.ds(off, P * D)].rearrange(
                                        "(p d) -> p d", d=D))
                                nc.vector.tensor_copy(
                                    Vs[:, QC + bi * QC + c, :, 0:DH],
                                    vt.rearrange("p (h dh) -> p h dh",
                                                 dh=DH))

                        # ---- remote pass: 12 key chunks per pair ----
                        NP_ = H // 2
                        probs_prev = None
                        cp_prev = None
                        prev_pair = None
                        for pair in range(NP_ + 1):
                            if pair < NP_:
                                probs_cur = [
                                    probsR.tile([P, 6, 2, NQ], U16,
                                                tag="pr", name=f"probs{h}")
                                    for h in range(2)]
                                cp_cur = ps_ctx.tile([P, 2, NQ], F32,
                                                     tag="cp", name="cp")
                            else:
                                cp_cur = None
                            for rc in range(12):
                                gc = QC + rc   # global chunk in rotation
                                if pair < NP_:
                                    sp = ps_sc.tile([P, 2, NQ], F32,
                                                    tag="sp", name="sp")
                                    for i in range(2):
                                        nc.tensor.matmul(
                                            sp[:, i, :],
                                            kT_all[i * DH:(i + 1) * DH,
                                                   pair,
                                                   gc * P:(gc + 1) * P],
                                            QT[i * DH:(i + 1) * DH, pair, :],
                                            start=True, stop=True)
                                    emit_exp(probs_cur[rc // 6], rc % 6,
                                             sp, gc,
                                             rc in DVE_SC_REMOTE, texpp)
                                if cp_prev is not None:
                                    hp = 2 * prev_pair
                                    for i in range(2):
                                        nc.tensor.matmul(
                                            cp_prev[0:DH + 2, i, :],
                                            Vs[:, QC + rc, hp + i, :],
                                            probs_prev[rc // 6][:, rc % 6,
                                                                i, :]
                                            .bitcast(BF16),
                                            start=(rc == 0), stop=(rc == 11))
                            if cp_prev is not None:
                                tt = smallB.tile([P, 2, NQ], F32, tag="tt",
                                                 name="tt")
                                nc.vector.tensor_tensor(
                                    tt[0:DH + 1, :, :],
                                    cp_prev[0:DH + 1, :, :],
                                    ctx_acc[0:DH + 1, prev_pair, :, :],
                                    OP.add)
                                rcp = smallB.tile([1, 2, NQ], F32,
                                                  tag="rcp", name="rcp")
                                nc.vector.reciprocal(rcp,
                                                     tt[DH:DH + 1, :, :])
                                rep = smallB.tile([DH, 2, NQ], F32,
                                                  tag="rep", name="rep")
                                nc.gpsimd.partition_broadcast(rep, rcp)
                                nc.vector.tensor_tensor(
                                    ctxT[0:DH, prev_pair, :], tt[0:DH, 0, :],
                                    rep[:, 0, :], OP.mult)
                                nc.vector.tensor_tensor(
                                    ctxT[DH:2 * DH, prev_pair, :],
                                    tt[0:DH, 1, :], rep[:, 1, :], OP.mult)
                            cp_prev = cp_cur
                            probs_prev = probs_cur if pair < NP_ else None
                            prev_pair = pair

            # ======== phases C+D ========
            with tc.tile_pool(name="pD", bufs=1) as pD:
              # D-phase weights prefetched early (run behind phase C)
              wd_sb = pD.tile([P, FC, D], BF16)      # Wd resident for D2
              for ff in range(0, FC, 4):
                  nc.sync.dma_start(wd_sb[:, ff:ff + 4, :],
                                    WdT[:, ff:ff + 4, :])
              wi0 = [pD.tile([P, 4, NQ], BF16, tag="wi_g",
                             name=f"wi0_{h}", bufs=4) for h in range(2)]
              for h in range(2):
                  nc.sync.dma_start(wi0[h], WiT[0, :, 4 * h:4 * h + 4, :])

              # ======== phase C: out-proj + LN1 + transpose ========
              with (
                tc.tile_pool(name="pC", bufs=1) as pC,
                tc.tile_pool(name="qnatC", bufs=1) as qnatC,
                tc.tile_pool(name="repC", bufs=1) as repC,
                tc.tile_pool(name="lnC", bufs=2) as lnC,
                tc.tile_pool(name="a1bfC", bufs=2) as a1bfC,
                tc.tile_pool(name="identC", bufs=1) as identC,
                tc.tile_pool(name="psC", bufs=2, space="PSUM") as psC,
                tc.tile_pool(name="psT2", bufs=2, space="PSUM") as psT2,
              ):
                attn_res = pC.tile([P, QC, D], F32)   # attn+residual
                ident = identC.tile([P, P], BF16)
                make_identity(nc, ident)
                bo_r = rep_row(repC, bo, "bo_r")
                g1_r = rep_row(repC, g1, "g1_r")
                b1_r = rep_row(repC, b1, "b1_r")
                q_nat = qnatC.tile([P, QC, D], F32)
                nc.sync.dma_start(q_nat,
                                  xq.rearrange("(c p) d -> p c d", p=P))
                qbo = q_nat
                for qc in range(QC):
                    nc.gpsimd.tensor_tensor(qbo[:, qc, :], q_nat[:, qc, :],
                                            bo_r, OP.add)
                for qc in range(QC):
                    pp = psC.tile([P, 2, NQ], F32, tag="ppC", name="pp")
                    for pc_ in range(KC):
                        for hf in range(2):
                            nc.tensor.matmul(
                                pp[:, hf, :],
                                ctxT[:, pc_, qc * P:(qc + 1) * P],
                                wo_b[:, pc_, hf * 512:(hf + 1) * 512],
                                start=(pc_ == 0), stop=(pc_ == KC - 1))
                    nc.vector.tensor_tensor(
                        attn_res[:, qc, :],
                        pp[:, :, :].rearrange("p h q -> p (h q)"),
                        qbo[:, qc, :], OP.add)
                    layernorm(lnC, attn_res, qc, g1_r, b1_r,
                              attn1[:, qc, :], "C")
                    a1bf = a1bfC.tile([P, D], BF16, tag="a1bf", name="a1bf")
                    nc.scalar.copy(a1bf, attn1[:, qc, :])
                    pt = psT2.tile([P, KC, P], BF16, tag="ptr2", name="pt")
                    for dc in range(KC):
                        nc.tensor.transpose(
                            pt[:, dc, :], a1bf[:, dc * P:(dc + 1) * P],
                            ident)
                    nc.vector.tensor_copy(
                        attn1T[:, :, qc * P:(qc + 1) * P], pt)

              # ======== phase D: FFN ========
              with tc.tile_pool(name="repD", bufs=1) as repD, \
                 tc.tile_pool(name="interp", bufs=1) as interp, \
                 tc.tile_pool(name="epD", bufs=1) as epD, \
                 tc.tile_pool(name="lnD", bufs=1) as lnD:
                bd_r = rep_row(repD, bd, "bd_r")
                g2_r = rep_row(repD, g2, "g2_r")
                b2_r = rep_row(repD, b2, "b2_r")
                interT = interp.tile([P, FC, NQ], BF16)
                a1bd = attn1
                for qc in range(QC):
                    nc.gpsimd.tensor_tensor(a1bd[:, qc, :], attn1[:, qc, :],
                                            bd_r, OP.add)

                # D1: interT = gelu(Wi^T @ attn1^T + bi), 4-col groups
                with tc.tile_pool(name="psD1", bufs=2, space="PSUM") as psD1:
                    for dg in range(DG):
                        if dg == 0:
                            wi_h = wi0
                        else:
                            wi_h = [pD.tile([P, 4, NQ], BF16, tag="wi_g",
                                            name=f"wi_h{h}", bufs=4)
                                    for h in range(2)]
                            for h in range(2):
                                nc.sync.dma_start(
                                    wi_h[h], WiT[dg, :, 4 * h:4 * h + 4, :])
                        ppg = [psD1.tile([P, NQ], F32, tag=f"ppD1_{j}",
                                         name=f"ppD1_{j}")
                               for j in range(4)]
                        for kc in range(KC):
                            for j in range(4):
                                nc.tensor.matmul(
                                    ppg[j],
                                    wi_h[kc // 4][:, kc % 4,
                                                  j * P:(j + 1) * P],
                                    attn1T[:, kc, :],
                                    start=(kc == 0), stop=(kc == KC - 1))
                        for j in range(4):
                            dc = dg * 4 + j
                            nc.scalar.activation(
                                interT[:, dc, :], ppg[j], AF.Gelu,
                                bias=bi_p[:, dc:dc + 1])

                # D2: layer_out = interT^T @ Wd + bd; +attn1; LN2.
                # qc-outer so LN2 + the output store pipeline behind MMs.
                layer_res = interp.tile([P, QC, D], F32, name="layer_res")
                out_v = out.rearrange("(c p) d -> p c d", p=P)
                with tc.tile_pool(name="psD2", bufs=2, space="PSUM") as psD2:
                    for qc in range(QC):
                        pp = psD2.tile([P, 2, NQ], F32, tag="ppD2",
                                       name="pp")
                        for kc2 in range(FC):
                            for hf in range(2):
                                nc.tensor.matmul(
                                    pp[:, hf, :],
                                    interT[:, kc2, qc * P:(qc + 1) * P],
                                    wd_sb[:, kc2, hf * 512:(hf + 1) * 512],
                                    start=(kc2 == 0),
                                    stop=(kc2 == FC - 1))
                        nc.vector.tensor_tensor(
                            layer_res[:, qc, :],
                            pp[:, :, :].rearrange("p h q -> p (h q)"),
                            a1bd[:, qc, :], OP.add)
                        o_t = epD.tile([P, D], F32, tag="o_t", name="o_t")
                        layernorm(lnD, layer_res, qc, g2_r, b2_r, o_t, "D")
                        nc.scalar.dma_start(out_v[:, qc, :], o_t)
            persBC_cm.__exit__(None, None, None)
            pCD_cm.__exit__(None, None, None)

    nc.compile()
    return nc


def _get_program():
    if "nc" not in _CACHE:
        _CACHE["nc"] = _build()
    return _CACHE["nc"]


def _prep_shared(inputs):
    def f32(x):
        return np.ascontiguousarray(np.asarray(x), dtype=np.float32)

    def bf(x):
        return np.ascontiguousarray(np.asarray(x, dtype=NPBF))

    Wq, Wk, Wv, Wo = (f32(inputs[n]) for n in ["Wq", "Wk", "Wv", "Wo"])
    Wi, Wd = f32(inputs["Wi"]), f32(inputs["Wd"])

    def tile_sq(w):  # [D, D] -> [P, KC, D]
        return bf(w.reshape(KC, P, D).transpose(1, 0, 2))

    shared = {
        "WqT": tile_sq(Wq), "WkT": tile_sq(Wk),
        "WvT": tile_sq(Wv), "WoT": tile_sq(Wo),
        # Wi [D, DFF] -> [DG, P, KC, NQ]: (d=kc*P+p, f=dg*NQ+j)
        "WiT": bf(Wi.reshape(KC, P, DG, NQ).transpose(2, 1, 0, 3)),
        # Wd [DFF, D] -> [P, FC, D]: (f=fc*P+p)
        "WdT": bf(Wd.reshape(FC, P, D).transpose(1, 0, 2)),
    }
    for n in ["bq", "bk", "bv", "bo", "bi", "bd",
              "ln1_g", "ln1_b", "ln2_g", "ln2_b"]:
        shared[n] = f32(inputs[n])
    return shared


def _run(inputs, trace=False):
    nc = _get_program()

    def f32(x):
        return np.ascontiguousarray(np.asarray(x), dtype=np.float32)

    def pick(*names):
        for n in names:
            if n in inputs:
                return inputs[n]
        raise KeyError(names[0])

    q = f32(pick("query"))
    k = f32(pick("key_in", "key"))
    v = f32(pick("value_in", "value"))
    m = f32(pick("attention_mask", "mask"))
    shared = _prep_shared(inputs)

    def xpose_tile(x_slice):  # [n, D] fp32 -> [P, KC, n] bf16
        n = x_slice.shape[0]
        xT = x_slice.T.astype(NPBF)           # [D, n]
        return np.ascontiguousarray(
            xT.reshape(KC, P, n).transpose(1, 0, 2))

    in_maps = []
    for c in range(8):
        b, r = c // 4, c % 4
        sl = slice(r * NQ, (r + 1) * NQ)
        im = dict(shared)
        im["xqT"] = xpose_tile(q[b, sl])
        im["xvT"] = xpose_tile(v[b, sl])
        im["xq"] = np.ascontiguousarray(q[b, sl])
        # full-batch keys + mask in rotation order (own block first)
        kp = np.concatenate(
            [k[b, ((r + j) % 4) * NQ:((r + j) % 4) * NQ + NQ]
             for j in range(4)], axis=0)
        im["xkTF"] = xpose_tile(kp)
        mrow = m[b, 0, 0, :]
        perm = np.concatenate(
            [mrow[((r + j) % 4) * NQ:((r + j) % 4) * NQ + NQ]
             for j in range(4)])
        im["mask"] = np.ascontiguousarray(perm, dtype=np.float32)
        im["magicm"] = np.ascontiguousarray(
            perm * C_LOG2E128 + (C_ANCHOR + C_BIAS), dtype=np.float32)
        in_maps.append(im)

    res = run_bass_kernel_spmd(nc, in_maps, core_ids=list(range(8)),
                               trace=trace)
    full = np.empty((B, S, D), dtype=np.float32)
    for c in range(8):
        b, r = c // 4, c % 4
        full[b, r * NQ:(r + 1) * NQ, :] = res.results[c]["out"]
    return full, res


def kernel(**inputs):
    full, _ = _run(inputs)
    return full


# revision 19
# speedup vs baseline: 1.0781x; 1.0781x over previous
"""BERT layer forward (nn_BertLayerForDecoder) on 8 trn2 NeuronCores.

Sharding: sequence-parallel. The (B=2, S=2048) = 4096 token rows are split
into 8 slices of 512 rows; core c owns rows [r*512, (r+1)*512) of batch
b = c // 4, r = c % 4. Q and V are projected per-slice; V is AllGathered
within each 4-core batch group (1 MB/rank). K^T is computed fully
per-core (each core holds the whole batch-slice key input): the own-block
K projection runs first so a "local pass" of attention over the own 512
keys starts immediately, and the remaining 3 blocks' K projections are
exactly the compute that hides the V gather. K^T stays resident in SBUF
(no HBM roundtrip). Key chunks are processed in rotation order (own
block first); the attention mask and the key input are host-permuted to
match, and the gathered V blocks are addressed with partition_id()-based
dynamic DMA offsets.

Softmax: no max-subtraction (scores are O(1)); mask add + 1/sqrt(dh)
scale folded into the exp. The exp work is split between the ACT engine
(exact spline exp) and the Vector engine (Schraudolph exponent-stuffing:
one fused multiply-add against a 2^23*1.5 anchor leaves the bf16 bit
pattern of exp(x) in the low fp32 mantissa bits, extracted with a
strided u16 copy). The softmax denominator comes from ones-columns
appended to V, so it falls out of the ctx matmul.

Numerics: matmul operands bf16 (fp32 PSUM accumulation), vector math in
fp32. DVE-exp probs carry ~3% relative error; the attention branch is
~1% of the output signal (the residual dominates), so the end-to-end
impact is ~1e-4.

Self-contained: hardcodes all shapes; only needs numpy + ml_dtypes + the
installed concourse package.
"""

import ml_dtypes
import numpy as np

import concourse.bacc as bacc
import concourse.bass as cbass
import concourse.mybir as mybir
import concourse.tile as tile
from concourse.bass_utils import run_bass_kernel_spmd
from concourse.masks import make_identity

F32 = mybir.dt.float32
BF16 = mybir.dt.bfloat16
U16 = mybir.dt.uint16
AF = mybir.ActivationFunctionType
OP = mybir.AluOpType
NPBF = ml_dtypes.bfloat16

B, S, D, H, DH, DFF = 2, 2048, 1024, 16, 64, 4096
P = 128
NQ = 512              # query rows per core
QC = NQ // P          # 4 q-chunks
KC = D // P           # 8 d-chunks (contraction)
SC = S // P           # 16 key chunks
FC = DFF // P         # 32 dff chunks
DG = FC // 4          # 8 ffn-up column groups (512 cols each)
EPS = 1e-12
KV_V = NQ * D          # bf16 elements gathered per rank (V rows)

# Schraudolph exp in bf16-bit space: with t = s*C_EXP + (mask*C_LOG2E128
# + C_BIAS + C_ANCHOR), the low 16 bits of fp32(t) hold bf16(exp(s*0.125
# + mask)). C_EXP folds the 1/sqrt(dh) attention scale.
C_LOG2E128 = 128 * 1.4426950408889634
C_EXP = 0.125 * C_LOG2E128
C_BIAS = 16256.0 - 5.590656            # 128*(127 - 0.043677)
C_ANCHOR = 12582912.0                  # 2^23 * 1.5
# which chunk indices of each pair-iteration run exp on the DVE (rest
# on ACT); tuned so the two engines finish together.
DVE_SC_LOCAL = (1, 3)
DVE_SC_REMOTE = (1, 4, 7, 10)

_CACHE = {}


def _build():
    nc = bacc.Bacc()

    # activations (own 512-row slice, pre-transposed bf16), except the
    # key input which is the full 2048-row batch slice in rotation order
    xqT = nc.declare_dram_parameter("xqT", [P, KC, NQ], BF16, isOutput=False)
    xkTF = nc.declare_dram_parameter("xkTF", [P, KC, S], BF16,
                                     isOutput=False)
    xvT = nc.declare_dram_parameter("xvT", [P, KC, NQ], BF16, isOutput=False)
    xq = nc.declare_dram_parameter("xq", [NQ, D], F32, isOutput=False)
    # mask/magic host-permuted into this core's rotation chunk order
    msk = nc.declare_dram_parameter("mask", [S], F32, isOutput=False)
    mgk = nc.declare_dram_parameter("magicm", [S], F32, isOutput=False)
    # weights: bf16, pre-tiled
    WqT = nc.declare_dram_parameter("WqT", [P, KC, D], BF16, isOutput=False)
    WkT = nc.declare_dram_parameter("WkT", [P, KC, D], BF16, isOutput=False)
    WvT = nc.declare_dram_parameter("WvT", [P, KC, D], BF16, isOutput=False)
    WoT = nc.declare_dram_parameter("WoT", [P, KC, D], BF16, isOutput=False)
    WiT = nc.declare_dram_parameter("WiT", [DG, P, KC, NQ], BF16,
                                    isOutput=False)
    WdT = nc.declare_dram_parameter("WdT", [P, FC, D], BF16, isOutput=False)
    bq = nc.declare_dram_parameter("bq", [D], F32, isOutput=False)
    bk = nc.declare_dram_parameter("bk", [D], F32, isOutput=False)
    bv = nc.declare_dram_parameter("bv", [D], F32, isOutput=False)
    bo = nc.declare_dram_parameter("bo", [D], F32, isOutput=False)
    bi = nc.declare_dram_parameter("bi", [DFF], F32, isOutput=False)
    bd = nc.declare_dram_parameter("bd", [D], F32, isOutput=False)
    g1 = nc.declare_dram_parameter("ln1_g", [D], F32, isOutput=False)
    b1 = nc.declare_dram_parameter("ln1_b", [D], F32, isOutput=False)
    g2 = nc.declare_dram_parameter("ln2_g", [D], F32, isOutput=False)
    b2 = nc.declare_dram_parameter("ln2_b", [D], F32, isOutput=False)
    out = nc.declare_dram_parameter("out", [NQ, D], F32, isOutput=True)

    # V collective bounce buffers (bf16); per block V is [NQ, D] s-major
    kvLb = nc.dram_tensor("kv_loc", [KV_V], BF16)
    kvAb = nc.dram_tensor("kv_all", [4 * KV_V], BF16)
    kvL_v = kvLb[:].rearrange("(s d) -> s d", d=D)

    with tile.TileContext(nc) as tc:
        with tc.tile_pool(name="const", bufs=1) as const:
            # ---------- small constants (resident) ----------
            mask_sb = const.tile([P, SC], F32)
            nc.gpsimd.dma_start(mask_sb, msk.rearrange("(c p) -> p c", p=P))
            magic_sb = const.tile([P, SC], F32)
            nc.gpsimd.dma_start(magic_sb, mgk.rearrange("(c p) -> p c", p=P))
            bq_p = const.tile([P, KC], F32)
            nc.gpsimd.dma_start(bq_p, bq.rearrange("(c p) -> p c", p=P))
            bk_p = const.tile([P, KC], F32)
            nc.gpsimd.dma_start(bk_p, bk.rearrange("(c p) -> p c", p=P))
            bi_p = const.tile([P, FC], F32)
            nc.gpsimd.dma_start(bi_p, bi.rearrange("(c p) -> p c", p=P))
            eps_sb = const.tile([P, 1], F32)
            nc.vector.memset(eps_sb, EPS)
            bv_r = const.tile([P, D], F32)
            nc.gpsimd.dma_start(bv_r,
                                bv.ap().unsqueeze(0).to_broadcast((P, D)))

            def rep_row(pool, vec, name):
                t = pool.tile([P, D], F32, tag=name, name=name)
                nc.sync.dma_start(t, vec.ap().unsqueeze(0).to_broadcast((P, D)))
                return t

            def layernorm(pool, x_res, qc, g_r, b_r, dst_ap, sfx):
                """mean/var via bn_stats; the [P, D] affine passes split
                across gpsimd/vector so neither serializes the tail."""
                st6 = pool.tile([P, 2, 6], F32, tag="st6" + sfx, name="st6")
                for j in range(2):
                    nc.vector.bn_stats(
                        st6[:, j, :], x_res[:, qc, j * 512:(j + 1) * 512])
                mv = pool.tile([P, 2], F32, tag="mv" + sfx, name="mv")
                nc.vector.bn_aggr(mv, st6)
                sq = pool.tile([P, 1], F32, tag="sq" + sfx, name="sq")
                nc.scalar.activation(sq, mv[:, 1:2], AF.Sqrt, bias=eps_sb)
                rstd = pool.tile([P, 1], F32, tag="rstd" + sfx, name="rstd")
                nc.vector.reciprocal(rstd, sq)
                xn = pool.tile([P, D], F32, tag="xn" + sfx, name="xn")
                nc.vector.tensor_scalar(
                    xn, x_res[:, qc, :], mv[:, 0:1], rstd,
                    OP.subtract, OP.mult)
                xg = pool.tile([P, D], F32, tag="xg" + sfx, name="xg")
                nc.vector.tensor_tensor(xg, xn, g_r, OP.mult)
                nc.vector.tensor_tensor(dst_ap, xg, b_r, OP.add)

            pCD_cm = tc.tile_pool(name="pCD", bufs=1)
            pCD = pCD_cm.__enter__()
            attn1 = pCD.tile([P, QC, D], F32)      # LN1 out (residual)
            attn1T = pCD.tile([P, KC, NQ], BF16)
            persBC_cm = tc.tile_pool(name="persBC", bufs=1)
            persBC = persBC_cm.__enter__()
            ctxT = persBC.tile([P, KC, NQ], BF16)      # ctx^T (dh-pairs, q)
            wo_b = persBC.tile([P, KC, D], BF16)       # Wo (loaded early)

            with tc.tile_pool(name="persB", bufs=1) as persB:
                QT = persB.tile([P, KC, NQ], BF16)     # Q^T
                kT_all = persB.tile([P, KC, S], BF16)  # all keys^T, resident
                Vs = persB.tile([P, SC, H, DH + 2], BF16)  # V + ones cols
                ctx_acc = persB.tile([P, KC, 2, NQ], BF16)  # local-pass ctx

                # ======== phase A: projections; V gather ========
                with (
                    tc.tile_pool(name="xT", bufs=2) as xT,
                    tc.tile_pool(name="wA", bufs=2) as wA,
                    tc.tile_pool(name="vsbA", bufs=1) as vsbA,
                    tc.tile_pool(name="psA", bufs=1, space="PSUM") as psA,
                ):
                    # --- own-block K projection (kc-major) ---
                    keyT = xT.tile([P, KC, NQ], BF16, tag="xpt", name="keyT")
                    wk_b = wA.tile([P, KC, D], BF16, tag="wk", name="wk_b",
                                   bufs=1)
                    for kk in range(0, KC, 2):
                        nc.sync.dma_start(keyT[:, kk:kk + 2, :],
                                          xkTF[:, kk:kk + 2, 0:NQ])
                        nc.sync.dma_start(wk_b[:, kk:kk + 2, :],
                                          WkT[:, kk:kk + 2, :])
                    ppK = [psA.tile([P, NQ], F32, tag=f"psA{j}",
                                    name=f"ppK{j}") for j in range(KC)]
                    for kc in range(KC):
                        for dc in range(KC):
                            nc.tensor.matmul(
                                ppK[dc], wk_b[:, kc, dc * P:(dc + 1) * P],
                                keyT[:, kc, :],
                                start=(kc == 0), stop=(kc == KC - 1))
                    for dc in range(KC):
                        if dc % 2 == 0:
                            nc.scalar.add(kT_all[:, dc, 0:NQ], ppK[dc],
                                          bk_p[:, dc:dc + 1])
                        else:
                            nc.vector.tensor_scalar_add(
                                kT_all[:, dc, 0:NQ], ppK[dc],
                                bk_p[:, dc:dc + 1])

                    # --- V projection -> gather ---
                    v_sb = vsbA.tile([P, QC, D], BF16)
                    valT = xT.tile([P, KC, NQ], BF16, tag="xpt", name="valT")
                    wv_b = wA.tile([P, KC, D], BF16, tag="wv", name="wv_b",
                                   bufs=1)
                    for kk in range(0, KC, 2):
                        nc.sync.dma_start(valT[:, kk:kk + 2, :],
                                          xvT[:, kk:kk + 2, :])
                        nc.sync.dma_start(wv_b[:, kk:kk + 2, :],
                                          WvT[:, kk:kk + 2, :])
                    ppV = [psA.tile([P, NQ], F32, tag=f"psA{j}",
                                    name=f"ppV{j}") for j in range(KC)]
                    for kc in range(KC):
                        for sl in range(KC):
                            sc4, hf = sl // 2, sl % 2
                            nc.tensor.matmul(
                                ppV[sl], valT[:, kc, sc4 * P:(sc4 + 1) * P],
                                wv_b[:, kc, hf * 512:(hf + 1) * 512],
                                start=(kc == 0), stop=(kc == KC - 1))
                    for sl in range(KC):
                        sc4, hf = sl // 2, sl % 2
                        nc.vector.tensor_tensor(
                            v_sb[:, sc4, hf * 512:(hf + 1) * 512], ppV[sl],
                            bv_r[:, hf * 512:(hf + 1) * 512], OP.add)
                    for sc4 in range(QC):
                        nc.scalar.dma_start(
                            kvL_v[sc4 * P:(sc4 + 1) * P, :], v_sb[:, sc4, :])

                    nc.gpsimd.collective_compute(
                        "AllGather", OP.bypass,
                        replica_groups=[[0, 1, 2, 3], [4, 5, 6, 7]],
                        ins=[kvLb[:]], outs=[kvAb[:]])

                    # --- Q projection (overlaps the gather) ---
                    qryT = xT.tile([P, KC, NQ], BF16, tag="xpt", name="qryT")
                    wq_b = wA.tile([P, KC, D], BF16, tag="wv", name="wq_b",
                                   bufs=1)
                    for kk in range(0, KC, 2):
                        nc.sync.dma_start(qryT[:, kk:kk + 2, :],
                                          xqT[:, kk:kk + 2, :])
                        nc.sync.dma_start(wq_b[:, kk:kk + 2, :],
                                          WqT[:, kk:kk + 2, :])
                    ppQ = [psA.tile([P, NQ], F32, tag=f"psA{j}",
                                    name=f"ppQ{j}") for j in range(KC)]
                    for kc in range(KC):
                        for dc in range(KC):
                            nc.tensor.matmul(
                                ppQ[dc], wq_b[:, kc, dc * P:(dc + 1) * P],
                                qryT[:, kc, :],
                                start=(kc == 0), stop=(kc == KC - 1))
                    for dc in range(KC):
                        nc.vector.tensor_scalar_add(
                            QT[:, dc, :], ppQ[dc], bq_p[:, dc:dc + 1])

                    # own V rows -> Vs rotation slots 0..3 (zero DMA)
                    nc.gpsimd.memset(Vs[:, :, :, DH:DH + 2], 1.0)
                    for c in range(QC):
                        nc.vector.tensor_copy(
                            Vs[:, c, :, 0:DH],
                            v_sb[:, c, :].rearrange("p (h dh) -> p h dh",
                                                    dh=DH))

                    # Wo prefetch (used in phase C)
                    for kk in range(0, KC, 4):
                        nc.sync.dma_start(wo_b[:, kk:kk + 4, :],
                                          WoT[:, kk:kk + 4, :])

                    # --- remote-block K projections: the gather cover ---
                    for bi2 in range(3):
                        keyR = xT.tile([P, KC, NQ], BF16, tag="xpt",
                                       name="keyR")
                        col = (1 + bi2) * NQ
                        for kk in range(0, KC, 2):
                            nc.sync.dma_start(
                                keyR[:, kk:kk + 2, :],
                                xkTF[:, kk:kk + 2, col:col + NQ])
                        ppR = [psA.tile([P, NQ], F32, tag=f"psA{j}",
                                        name=f"ppR{j}") for j in range(KC)]
                        for kc in range(KC):
                            for dc in range(KC):
                                nc.tensor.matmul(
                                    ppR[dc],
                                    wk_b[:, kc, dc * P:(dc + 1) * P],
                                    keyR[:, kc, :],
                                    start=(kc == 0), stop=(kc == KC - 1))
                        for dc in range(KC):
                            if dc % 2 == 0:
                                nc.scalar.add(kT_all[:, dc, col:col + NQ],
                                              ppR[dc], bk_p[:, dc:dc + 1])
                            else:
                                nc.vector.tensor_scalar_add(
                                    kT_all[:, dc, col:col + NQ], ppR[dc],
                                    bk_p[:, dc:dc + 1])

                # ======== phase B: attention ========
                def emit_exp(probs_t, ci, sp, col, on_dve, scratch):
                    """exp of [P, 2, NQ] scores -> bf16 probs chunk ci of
                    the u16 probs tile [P, n, 2, NQ]."""
                    if on_dve:
                        texp = scratch.tile([P, 2, NQ], F32, tag="texp",
                                            name="texp")
                        nc.vector.tensor_scalar(
                            texp, sp, C_EXP, magic_sb[:, col:col + 1],
                            OP.mult, OP.add)
                        tv = texp[:, :, :].bitcast(U16).rearrange(
                            "p h (q t) -> p t h q", t=2)
                        nc.vector.tensor_copy(
                            probs_t[:, ci:ci + 1, :, :], tv[:, 0:1, :, :])
                    else:
                        nc.scalar.activation(
                            probs_t[:, ci, :, :].bitcast(BF16), sp, AF.Exp,
                            bias=mask_sb[:, col:col + 1], scale=0.125)

                with (
                    tc.tile_pool(name="texpp", bufs=2) as texpp,
                    tc.tile_pool(name="smallB", bufs=2) as smallB,
                    tc.tile_pool(name="probsP", bufs=4) as probsP,
                    tc.tile_pool(name="ps_sc", bufs=2, space="PSUM") as ps_sc,
                    tc.tile_pool(name="ps_ctx", bufs=2,
                                 space="PSUM") as ps_ctx,
                ):
                    # ---- local pass: own 4 key chunks, ctx lags scores
                    # by 2 chunks so the exp latency is hidden ----
                    for pair in range(H // 2):
                        pr = {}
                        cp = ps_ctx.tile([P, 2, NQ], F32, tag="cp",
                                         name="cp")
                        for c in range(QC + 2):
                            if c < QC:
                                sp = ps_sc.tile([P, 2, NQ], F32, tag="sp",
                                                name="sp")
                                for i in range(2):
                                    nc.tensor.matmul(
                                        sp[:, i, :],
                                        kT_all[i * DH:(i + 1) * DH, pair,
                                               c * P:(c + 1) * P],
                                        QT[i * DH:(i + 1) * DH, pair, :],
                                        start=True, stop=True)
                                pt_ = probsP.tile([P, 1, 2, NQ], U16,
                                                  tag="pp", name="pr")
                                pr[c] = pt_
                                emit_exp(pt_, 0, sp, c,
                                         c in DVE_SC_LOCAL, texpp)
                            if c >= 2:
                                cc = c - 2
                                for i in range(2):
                                    nc.tensor.matmul(
                                        cp[0:DH + 2, i, :],
                                        Vs[:, cc, 2 * pair + i, :],
                                        pr[cc][:, 0, i, :].bitcast(BF16),
                                        start=(cc == 0),
                                        stop=(cc == QC - 1))
                        nc.vector.tensor_copy(
                            ctx_acc[0:DH + 1, pair, :, :],
                            cp[0:DH + 1, :, :])

                    # ---- remote Vs assembly (rotation order) ----
                    own_s = nc.sync.partition_id()
                    blk_s = [nc.sync.snap(((own_s & 3) + 1 + bi) & 3,
                                          min_val=0, max_val=3)
                             for bi in range(3)]
                    with (
                        tc.tile_pool(name="vstr", bufs=2) as vstr,
                    ):
                        for bi in range(3):
                            for c in range(QC):
                                vt = vstr.tile([P, D], BF16, tag="vstr",
                                               name="vt")
                                off = blk_s[bi] * KV_V + c * P * D
                                nc.sync.dma_start(
                                    vt,
                                    kvAb[cbass.ds(off, P * D)].rearrange(
                                        "(p d) -> p d", d=D))
                                nc.vector.tensor_copy(
                                    Vs[:, QC + bi * QC + c, :, 0:DH],
                                    vt.rearrange("p (h dh) -> p h dh",
                                                 dh=DH))

                        # ---- remote pass: 12 chunks per pair, ctx lags
                        # scores by 2 chunks; normalize at pair end ----
                        NP_ = H // 2
                        for pair in range(NP_):
                            pr = {}
                            cp = ps_ctx.tile([P, 2, NQ], F32, tag="cp",
                                             name="cp")
                            for rc in range(14):
                                gc = QC + rc
                                if rc < 12:
                                    sp = ps_sc.tile([P, 2, NQ], F32,
                                                    tag="sp", name="sp")
                                    for i in range(2):
                                        nc.tensor.matmul(
                                            sp[:, i, :],
                                            kT_all[i * DH:(i + 1) * DH,
                                                   pair,
                                                   gc * P:(gc + 1) * P],
                                            QT[i * DH:(i + 1) * DH, pair, :],
                                            start=True, stop=True)
                                    pt_ = probsP.tile([P, 1, 2, NQ], U16,
                                                      tag="pp", name="pr")
                                    pr[rc] = pt_
                                    emit_exp(pt_, 0, sp, gc,
                                             rc in DVE_SC_REMOTE, texpp)
                                if rc >= 2:
                                    cc = rc - 2
                                    for i in range(2):
                                        nc.tensor.matmul(
                                            cp[0:DH + 2, i, :],
                                            Vs[:, QC + cc, 2 * pair + i, :],
                                            pr[cc][:, 0, i, :]
                                            .bitcast(BF16),
                                            start=(cc == 0), stop=(cc == 11))
                            tt = smallB.tile([P, 2, NQ], F32, tag="tt",
                                             name="tt")
                            nc.vector.tensor_tensor(
                                tt[0:DH + 1, :, :],
                                cp[0:DH + 1, :, :],
                                ctx_acc[0:DH + 1, pair, :, :],
                                OP.add)
                            rcp = smallB.tile([1, 2, NQ], F32,
                                              tag="rcp", name="rcp")
                            nc.vector.reciprocal(rcp, tt[DH:DH + 1, :, :])
                            rep = smallB.tile([DH, 2, NQ], F32,
                                              tag="rep", name="rep")
                            nc.gpsimd.partition_broadcast(rep, rcp)
                            nc.vector.tensor_tensor(
                                ctxT[0:DH, pair, :], tt[0:DH, 0, :],
                                rep[:, 0, :], OP.mult)
                            nc.vector.tensor_tensor(
                                ctxT[DH:2 * DH, pair, :],
                                tt[0:DH, 1, :], rep[:, 1, :], OP.mult)

            # ======== phases C+D ========
            with tc.tile_pool(name="pD", bufs=1) as pD:
              # D-phase weights prefetched early (run behind phase C)
              wd_sb = pD.tile([P, FC, D], BF16)      # Wd resident for D2
              for ff in range(0, FC, 4):
                  nc.sync.dma_start(wd_sb[:, ff:ff + 4, :],
                                    WdT[:, ff:ff + 4, :])
              wi0 = [pD.tile([P, 4, NQ], BF16, tag="wi_g",
                             name=f"wi0_{h}", bufs=4) for h in range(2)]
              for h in range(2):
                  nc.sync.dma_start(wi0[h], WiT[0, :, 4 * h:4 * h + 4, :])

              # ======== phase C: out-proj + LN1 + transpose ========
              with (
                tc.tile_pool(name="pC", bufs=1) as pC,
                tc.tile_pool(name="qnatC", bufs=1) as qnatC,
                tc.tile_pool(name="repC", bufs=1) as repC,
                tc.tile_pool(name="lnC", bufs=2) as lnC,
                tc.tile_pool(name="a1bfC", bufs=2) as a1bfC,
                tc.tile_pool(name="identC", bufs=1) as identC,
                tc.tile_pool(name="psC", bufs=2, space="PSUM") as psC,
                tc.tile_pool(name="psT2", bufs=2, space="PSUM") as psT2,
              ):
                attn_res = pC.tile([P, QC, D], F32)   # attn+residual
                ident = identC.tile([P, P], BF16)
                make_identity(nc, ident)
                bo_r = rep_row(repC, bo, "bo_r")
                g1_r = rep_row(repC, g1, "g1_r")
                b1_r = rep_row(repC, b1, "b1_r")
                q_nat = qnatC.tile([P, QC, D], F32)
                nc.sync.dma_start(q_nat,
                                  xq.rearrange("(c p) d -> p c d", p=P))
                qbo = q_nat
                for qc in range(QC):
                    nc.vector.tensor_tensor(qbo[:, qc, :], q_nat[:, qc, :],
                                            bo_r, OP.add)
                for qc in range(QC):
                    pp = psC.tile([P, 2, NQ], F32, tag="ppC", name="pp")
                    for pc_ in range(KC):
                        for hf in range(2):
                            nc.tensor.matmul(
                                pp[:, hf, :],
                                ctxT[:, pc_, qc * P:(qc + 1) * P],
                                wo_b[:, pc_, hf * 512:(hf + 1) * 512],
                                start=(pc_ == 0), stop=(pc_ == KC - 1))
                    nc.vector.tensor_tensor(
                        attn_res[:, qc, :],
                        pp[:, :, :].rearrange("p h q -> p (h q)"),
                        qbo[:, qc, :], OP.add)
                    layernorm(lnC, attn_res, qc, g1_r, b1_r,
                              attn1[:, qc, :], "C")
                    a1bf = a1bfC.tile([P, D], BF16, tag="a1bf", name="a1bf")
                    nc.scalar.copy(a1bf, attn1[:, qc, :])
                    pt = psT2.tile([P, KC, P], BF16, tag="ptr2", name="pt")
                    for dc in range(KC):
                        nc.tensor.transpose(
                            pt[:, dc, :], a1bf[:, dc * P:(dc + 1) * P],
                            ident)
                    nc.vector.tensor_copy(
                        attn1T[:, :, qc * P:(qc + 1) * P], pt)

              # ======== phase D: FFN ========
              with tc.tile_pool(name="repD", bufs=1) as repD, \
                 tc.tile_pool(name="interp", bufs=1) as interp, \
                 tc.tile_pool(name="epD", bufs=1) as epD, \
                 tc.tile_pool(name="lnD", bufs=1) as lnD:
                bd_r = rep_row(repD, bd, "bd_r")
                g2_r = rep_row(repD, g2, "g2_r")
                b2_r = rep_row(repD, b2, "b2_r")
                interT = interp.tile([P, FC, NQ], BF16)
                a1bd = attn1
                for qc in range(QC):
                    nc.vector.tensor_tensor(a1bd[:, qc, :], attn1[:, qc, :],
                                            bd_r, OP.add)

                # D1: interT = gelu(Wi^T @ attn1^T + bi), 4-col groups
                with tc.tile_pool(name="psD1", bufs=2, space="PSUM") as psD1:
                    for dg in range(DG):
                        if dg == 0:
                            wi_h = wi0
                        else:
                            wi_h = [pD.tile([P, 4, NQ], BF16, tag="wi_g",
                                            name=f"wi_h{h}", bufs=4)
                                    for h in range(2)]
                            for h in range(2):
                                nc.sync.dma_start(
                                    wi_h[h], WiT[dg, :, 4 * h:4 * h + 4, :])
                        ppg = [psD1.tile([P, NQ], F32, tag=f"ppD1_{j}",
                                         name=f"ppD1_{j}")
                               for j in range(4)]
                        for kc in range(KC):
                            for j in range(4):
                                nc.tensor.matmul(
                                    ppg[j],
                                    wi_h[kc // 4][:, kc % 4,
                                                  j * P:(j + 1) * P],
                                    attn1T[:, kc, :],
                                    start=(kc == 0), stop=(kc == KC - 1))
                        for j in range(4):
                            dc = dg * 4 + j
                            nc.scalar.activation(
                                interT[:, dc, :], ppg[j], AF.Gelu,
                                bias=bi_p[:, dc:dc + 1])

                # D2: layer_out = interT^T @ Wd + bd; +attn1; LN2.
                # qc-outer so LN2 + the output store pipeline behind MMs.
                layer_res = interp.tile([P, QC, D], F32, name="layer_res")
                out_v = out.rearrange("(c p) d -> p c d", p=P)
                with tc.tile_pool(name="psD2", bufs=2, space="PSUM") as psD2:
                    for qc in range(QC):
                        pp = psD2.tile([P, 2, NQ], F32, tag="ppD2",
                                       name="pp")
                        for kc2 in range(FC):
                            for hf in range(2):
                                nc.tensor.matmul(
                                    pp[:, hf, :],
                                    interT[:, kc2, qc * P:(qc + 1) * P],
                                    wd_sb[:, kc2, hf * 512:(hf + 1) * 512],
                                    start=(kc2 == 0),
                                    stop=(kc2 == FC - 1))
                        nc.vector.tensor_tensor(
                            layer_res[:, qc, :],
                            pp[:, :, :].rearrange("p h q -> p (h q)"),
                            a1bd[:, qc, :], OP.add)
                        o_t = epD.tile([P, D], F32, tag="o_t", name="o_t")
                        layernorm(lnD, layer_res, qc, g2_r, b2_r, o_t, "D")
                        nc.scalar.dma_start(out_v[:, qc, :], o_t)
            persBC_cm.__exit__(None, None, None)
            pCD_cm.__exit__(None, None, None)

    nc.compile()
    return nc


def _get_program():
    if "nc" not in _CACHE:
        _CACHE["nc"] = _build()
    return _CACHE["nc"]


def _prep_shared(inputs):
    def f32(x):
        return np.ascontiguousarray(np.asarray(x), dtype=np.float32)

    def bf(x):
        return np.ascontiguousarray(np.asarray(x, dtype=NPBF))

    Wq, Wk, Wv, Wo = (f32(inputs[n]) for n in ["Wq", "Wk", "Wv", "Wo"])
    Wi, Wd = f32(inputs["Wi"]), f32(inputs["Wd"])

    def tile_sq(w):  # [D, D] -> [P, KC, D]
        return bf(w.reshape(KC, P, D).transpose(1, 0, 2))

    shared = {
        "WqT": tile_sq(Wq), "WkT": tile_sq(Wk),
        "WvT": tile_sq(Wv), "WoT": tile_sq(Wo),
        # Wi [D, DFF] -> [DG, P, KC, NQ]: (d=kc*P+p, f=dg*NQ+j)
        "WiT": bf(Wi.reshape(KC, P, DG, NQ).transpose(2, 1, 0, 3)),
        # Wd [DFF, D] -> [P, FC, D]: (f=fc*P+p)
        "WdT": bf(Wd.reshape(FC, P, D).transpose(1, 0, 2)),
    }
    for n in ["bq", "bk", "bv", "bo", "bi", "bd",
              "ln1_g", "ln1_b", "ln2_g", "ln2_b"]:
        shared[n] = f32(inputs[n])
    return shared


def _run(inputs, trace=False):
    nc = _get_program()

    def f32(x):
        return np.ascontiguousarray(np.asarray(x), dtype=np.float32)

    def pick(*names):
        for n in names:
            if n in inputs:
                return inputs[n]
        raise KeyError(names[0])

    q = f32(pick("query"))
    k = f32(pick("key_in", "key"))
    v = f32(pick("value_in", "value"))
    m = f32(pick("attention_mask", "mask"))
    shared = _prep_shared(inputs)

    def xpose_tile(x_slice):  # [n, D] fp32 -> [P, KC, n] bf16
        n = x_slice.shape[0]
        xT = x_slice.T.astype(NPBF)           # [D, n]
        return np.ascontiguousarray(
            xT.reshape(KC, P, n).transpose(1, 0, 2))

    in_maps = []
    for c in range(8):
        b, r = c // 4, c % 4
        sl = slice(r * NQ, (r + 1) * NQ)
        im = dict(shared)
        im["xqT"] = xpose_tile(q[b, sl])
        im["xvT"] = xpose_tile(v[b, sl])
        im["xq"] = np.ascontiguousarray(q[b, sl])
        # full-batch keys + mask in rotation order (own block first)
        kp = np.concatenate(
            [k[b, ((r + j) % 4) * NQ:((r + j) % 4) * NQ + NQ]
             for j in range(4)], axis=0)
        im["xkTF"] = xpose_tile(kp)
        mrow = m[b, 0, 0, :]
        perm = np.concatenate(
            [mrow[((r + j) % 4) * NQ:((r + j) % 4) * NQ + NQ]
             for j in range(4)])
        im["mask"] = np.ascontiguousarray(perm, dtype=np.float32)
        im["magicm"] = np.ascontiguousarray(
            perm * C_LOG2E128 + (C_ANCHOR + C_BIAS), dtype=np.float32)
        in_maps.append(im)

    res = run_bass_kernel_spmd(nc, in_maps, core_ids=list(range(8)),
                               trace=trace)
    full = np.empty((B, S, D), dtype=np.float32)
    for c in range(8):
        b, r = c // 4, c % 4
        full[b, r * NQ:(r + 1) * NQ, :] = res.results[c]["out"]
    return full, res


def kernel(**inputs):
    full, _ = _run(inputs)
    return full
